# revision 17
# baseline (speedup 1.0000x reference)
"""Int8 LLaMA MLP (SwiGLU, W8A8) on 8 TRN2 NeuronCores.

Two Bass programs:

1. PREP (runs once per kernel() call): weights arrive over the host link
   as int8 SHARDS (1/8 per core -> one full copy total on the wire instead
   of 8 replicated bf16 copies). On device: AllGather the int8 shards,
   cast int8->bf16 into the tiled layout the compute kernel wants, all
   kept resident in device HBM as jax arrays.

2. MAIN (the hot kernel, data-parallel over tokens, zero collectives):
   identical to the proven baseline. All GEMMs in bf16 (int8 values are
   exact in bf16; PSUM accumulates fp32 exactly). Transposed dataflow:
   stage-1 output G^T/U^T = [inter, tok] so the requantized Q^T feeds the
   down-proj directly as the moving operand -- no on-device transposes.

Per core: 2 token-chunks of 512.
  Phase A (per chunk): for each of 86 i-blocks: accumulate gate and up
    GEMMs over 32 h-blocks into PSUM, then SiLU/dequant (ACT) * dequant
    (ACT), clip, RNE-round via +/- 1.5*2^23, cast to bf16 into the
    SBUF-resident Q^T chunk.
  Phase B (per chunk): 4 sweeps of 8 output h-blocks; each sweep
    accumulates over all 86 i-blocks into 8 PSUM banks, then bias+scale
    (ACT) and DMA out. Output is Y^T [4096, 1024] fp32 per core; host
    transposes back.

Timing: all inputs are device-resident (device_put once); iterations
chain through donated output buffers, so steady-state per-iteration wall
time is dispatch + actual HW execution.
"""

import os
import time

import ml_dtypes
import numpy as np

import concourse.bass as bass
import concourse.mybir as mybir
import concourse.tile as tile
from concourse.bass_utils import run_bass_kernel_spmd  # noqa: F401 (env contract)

T, H, I = 8192, 4096, 11008
N_CORES = 8
TPC = T // N_CORES          # tokens per core = 1024
TC = 512                    # token chunk
N_CHUNK = TPC // TC         # 2
IB = I // 128               # 86 i-blocks
IBP = 88                    # padded i-blocks (88 = 8 * 11, AllGather-even)
IBS = IBP // N_CORES        # i-blocks per core shard = 11
IP = IBP * 128              # padded intermediate dim = 11264
IPS = IP // N_CORES         # down-weight rows per core shard = 1408
HK = H // 128               # 32 h-blocks (contraction for gate/up)
HB = H // 128               # 32 output h-blocks for down proj
HB_PER_SWEEP = 8            # PSUM banks used per down sweep
N_SWEEP = HB // HB_PER_SWEEP  # 4
IK_GRP = 4                  # i-blocks per down-weight DMA

MAGIC = float(1.5 * 2**23)  # fp32 round-to-nearest-even trick

BF16 = ml_dtypes.bfloat16

_exec_ns = None       # HW exec time (ns): NTFF profile if available, else wall
_exec_wall_ns = None  # chained-dispatch per-iteration wall (ns)
_exec_ntff_ns = None  # neuron-profile NTFF exec time (ns), when captured


# --------------------------------------------------------------------------
# Bass program 1: prep (AllGather int8 weight shards, cast to bf16 tiles)
# --------------------------------------------------------------------------
def _build_prep() -> bass.Bass:
    nc = bass.Bass(enable_partition_id=False, num_devices=N_CORES)
    dt = mybir.dt

    gus_d = nc.dram_tensor("gus", [IBS, HK, 128, 256], dt.int8,
                           kind="ExternalInput")
    dns_d = nc.dram_tensor("dns", [IPS, H], dt.int8, kind="ExternalInput")
    x8_d = nc.dram_tensor("x8", [N_CHUNK, HK, 128, TC], dt.int8,
                          kind="ExternalInput")

    gubf_d = nc.dram_tensor("gubf", [IB, HK, 128, 256], dt.bfloat16,
                            kind="ExternalOutput")
    dnbf_d = nc.dram_tensor("dnbf", [I, H], dt.bfloat16,
                            kind="ExternalOutput")
    xbf_d = nc.dram_tensor("xbf", [N_CHUNK, HK, 128, TC], dt.bfloat16,
                           kind="ExternalOutput")

    gus_st = nc.dram_tensor("gus_st", [IBS, HK, 128, 256], dt.int8)
    dns_st = nc.dram_tensor("dns_st", [IPS, H], dt.int8)
    gu_g = nc.dram_tensor("gu_g", [IBP, HK, 128, 256], dt.int8,
                          addr_space="Shared")
    dn_g = nc.dram_tensor("dn_g", [IP, H], dt.int8, addr_space="Shared")

    groups = [[i for i in range(N_CORES)]]

    with tile.TileContext(nc) as tc:
        with (
            tc.tile_pool(name="gp", bufs=3) as gp,
            tc.tile_pool(name="go", bufs=3) as go,
            tc.tile_pool(name="dp", bufs=3) as dp,
            tc.tile_pool(name="do", bufs=3) as do,
        ):
            # stage shards into internal DRAM (collectives can't touch I/O)
            nc.sync.dma_start(gus_st[:, :, :, :], gus_d[:, :, :, :])
            nc.sync.dma_start(dns_st[:, :], dns_d[:, :])
            nc.gpsimd.collective_compute(
                "AllGather", mybir.AluOpType.bypass, replica_groups=groups,
                ins=[gus_st[:, :, :, :].opt()], outs=[gu_g[:, :, :, :].opt()])
            nc.gpsimd.collective_compute(
                "AllGather", mybir.AluOpType.bypass, replica_groups=groups,
                ins=[dns_st[:, :].opt()], outs=[dn_g[:, :].opt()])

            # x cast (local, no collective): [128, 8, TC] slabs
            for ch in range(N_CHUNK):
                for k0 in range(0, HK, 8):
                    x_i8 = gp.tile([128, 8, TC], dt.int8, tag="gi")
                    nc.sync.dma_start(
                        x_i8,
                        x8_d[ch, k0: k0 + 8].rearrange("hk p t -> p hk t"))
                    x_bf = go.tile([128, 8, TC], dt.bfloat16, tag="go")
                    nc.vector.tensor_copy(x_bf, x_i8)
                    nc.sync.dma_start(
                        xbf_d[ch, k0: k0 + 8].rearrange("hk p t -> p hk t"),
                        x_bf)

            # gate/up: per i-block [128, HK, 256] tiles (DVE casts)
            for ib in range(IB):
                g_i8 = gp.tile([128, HK, 256], dt.int8, tag="gi")
                nc.sync.dma_start(g_i8, gu_g[ib].rearrange("hk p i -> p hk i"))
                g_bf = go.tile([128, HK, 256], dt.bfloat16, tag="go")
                nc.vector.tensor_copy(g_bf, g_i8)
                nc.sync.dma_start(gubf_d[ib].rearrange("hk p i -> p hk i"),
                                  g_bf)

            # down: 128-row groups [128, H] (ACT casts, runs parallel to DVE)
            for r in range(IB):
                d_i8 = dp.tile([128, H], dt.int8, tag="di")
                nc.sync.dma_start(d_i8, dn_g[r * 128: (r + 1) * 128, :])
                d_bf = do.tile([128, H], dt.bfloat16, tag="do")
                nc.scalar.copy(d_bf, d_i8)
                nc.sync.dma_start(dnbf_d[r * 128: (r + 1) * 128, :], d_bf)
    return nc


# --------------------------------------------------------------------------
# Bass program 2: main MLP kernel (identical to the proven baseline)
# --------------------------------------------------------------------------
def _build_main(gate_a: float, up_a: float, down_a: float,
                hb_per_sweep: int = HB_PER_SWEEP,
                dp_bufs: int = 2) -> bass.Bass:
    n_sweep = HB // hb_per_sweep
    nc = bass.Bass(enable_partition_id=False)
    dt = mybir.dt
    AF = mybir.ActivationFunctionType
    OP = mybir.AluOpType

    x_d = nc.dram_tensor("x", [N_CHUNK, HK, 128, TC], dt.bfloat16,
                         kind="ExternalInput")
    gu_d = nc.dram_tensor("gu", [IB, HK, 128, 256], dt.bfloat16,
                          kind="ExternalInput")
    dn_d = nc.dram_tensor("dn", [I, H], dt.bfloat16, kind="ExternalInput")
    gb_d = nc.dram_tensor("gb", [128, IB], dt.float32, kind="ExternalInput")
    ub_d = nc.dram_tensor("ub", [128, IB], dt.float32, kind="ExternalInput")
    db_d = nc.dram_tensor("db", [128, HB], dt.float32, kind="ExternalInput")
    out_d = nc.dram_tensor("out", [H, TPC], dt.float32, kind="ExternalOutput")

    with tile.TileContext(nc) as tc:
        with (
            tc.tile_pool(name="xp", bufs=1) as xp,
            tc.tile_pool(name="qp", bufs=1) as qp,
            tc.tile_pool(name="wp", bufs=2) as wp,
            tc.tile_pool(name="dp", bufs=dp_bufs) as dp,
            tc.tile_pool(name="tp", bufs=2) as tp,
            tc.tile_pool(name="yp", bufs=2) as yp,
            tc.tile_pool(name="bp", bufs=1) as bp,
            tc.tile_pool(name="ps", bufs=8, space="PSUM") as ps,
        ):
            gb_sb = bp.tile([128, IB], dt.float32)
            nc.sync.dma_start(gb_sb, gb_d[:, :])
            ub_sb = bp.tile([128, IB], dt.float32)
            nc.sync.dma_start(ub_sb, ub_d[:, :])
            db_sb = bp.tile([128, HB], dt.float32)
            nc.sync.dma_start(db_sb, db_d[:, :])

            for ch in range(N_CHUNK):
                # ---------------- Phase A: gate/up + SwiGLU + requant ----
                x_sb = xp.tile([128, HK, TC], dt.bfloat16, tag="x")
                nc.sync.dma_start(x_sb, x_d[ch].rearrange("hk p t -> p hk t"))
                q_sb = qp.tile([128, IB, TC], dt.bfloat16, tag="q")

                for ib in range(IB):
                    gu_sb = wp.tile([128, HK, 256], dt.bfloat16, tag="gu")
                    nc.sync.dma_start(
                        gu_sb, gu_d[ib].rearrange("hk p i -> p hk i"))
                    g_ps = ps.tile([128, TC], dt.float32, tag="ps")
                    u_ps = ps.tile([128, TC], dt.float32, tag="ps")
                    for hk in range(HK):
                        nc.tensor.matmul(
                            g_ps,
                            lhsT=gu_sb[:, hk, 0:128],
                            rhs=x_sb[:, hk, :],
                            start=(hk == 0), stop=(hk == HK - 1))
                        nc.tensor.matmul(
                            u_ps,
                            lhsT=gu_sb[:, hk, 128:256],
                            rhs=x_sb[:, hk, :],
                            start=(hk == 0), stop=(hk == HK - 1))
                    # s = silu(g*a + b); u = u*a + b
                    s_sb = tp.tile([128, TC], dt.float32, tag="s")
                    nc.scalar.activation(s_sb, g_ps, AF.Silu,
                                         bias=gb_sb[:, ib: ib + 1],
                                         scale=gate_a)
                    u_sb = tp.tile([128, TC], dt.float32, tag="u")
                    nc.scalar.activation(u_sb, u_ps, AF.Identity,
                                         bias=ub_sb[:, ib: ib + 1],
                                         scale=up_a)
                    p_sb = tp.tile([128, TC], dt.float32, tag="p")
                    nc.vector.tensor_mul(p_sb, s_sb, u_sb)
                    # clip first (clip-then-round == round-then-clip here),
                    # then RNE-round via +/- 1.5*2^23
                    c_sb = tp.tile([128, TC], dt.float32, tag="s")
                    nc.vector.tensor_scalar(c_sb, p_sb, -128.0, 127.0,
                                            OP.max, OP.min)
                    t_sb = tp.tile([128, TC], dt.float32, tag="u")
                    nc.vector.tensor_scalar_add(t_sb, c_sb, MAGIC)
                    nc.vector.tensor_scalar_sub(q_sb[:, ib, :], t_sb, MAGIC)

                # ---------------- Phase B: down proj --------------------
                for sw in range(n_sweep):
                    y_ps = [ps.tile([128, TC], dt.float32, tag="ps",
                                    name=f"y{ch}_{sw}_{hb}")
                            for hb in range(hb_per_sweep)]
                    for i0 in range(0, IB, IK_GRP):
                        g = min(IK_GRP, IB - i0)
                        dn_sb = dp.tile([128, IK_GRP, hb_per_sweep * 128],
                                        dt.bfloat16, tag="dn")
                        nc.sync.dma_start(
                            dn_sb[:, :g, :],
                            dn_d[i0 * 128: (i0 + g) * 128,
                                 sw * hb_per_sweep * 128:
                                 (sw + 1) * hb_per_sweep * 128]
                            .rearrange("(ik p) h -> p ik h", p=128))
                        for ik in range(g):
                            i_k = i0 + ik
                            rhs = q_sb[:, i_k, :]
                            for hb in range(hb_per_sweep):
                                nc.tensor.matmul(
                                    y_ps[hb],
                                    lhsT=dn_sb[:, ik, hb * 128: (hb + 1) * 128],
                                    rhs=rhs,
                                    start=(i_k == 0), stop=(i_k == IB - 1))
                    for hb in range(hb_per_sweep):
                        hg = sw * hb_per_sweep + hb
                        y_sb = yp.tile([128, TC], dt.float32, tag="y")
                        nc.scalar.activation(y_sb, y_ps[hb], AF.Identity,
                                             bias=db_sb[:, hg: hg + 1],
                                             scale=down_a)
                        nc.sync.dma_start(
                            out_d[hg * 128: (hg + 1) * 128,
                                  ch * TC: (ch + 1) * TC], y_sb)
    return nc


def _split_waits(nc):
    """Walrus in this container allows only ONE sync-wait per engine
    instruction (setupSyncWait capacity). Hoist extra waits onto injected
    same-engine NOPs (in-order engines -> semantics unchanged)."""
    for fn in nc.m.functions:
        for bb in fn.blocks:
            out = []
            for inst in bb.instructions:
                si = inst.sync_info
                if si is not None and si.on_wait and len(si.on_wait) > 1:
                    waits = list(si.on_wait)
                    for j, w in enumerate(waits[:-1]):
                        nop = mybir.InstNoOp(name=f"{inst.name}-w{j}",
                                             ins=[], outs=[])
                        nop.engine = inst.engine
                        nop.sync_info = mybir.SyncInfo(on_wait=[w],
                                                       on_update=[])
                        out.append(nop)
                    si.on_wait = [waits[-1]]
                out.append(inst)
            bb.instructions = out


# --------------------------------------------------------------------------
# Host-side input prep: int32 -> int8, tile, shard. All cheap int8 ops.
# --------------------------------------------------------------------------
def _prep_inputs(hidden_states, gate_w, gate_b, up_w, up_b, down_w, down_b):
    gate_i8 = np.asarray(gate_w, dtype=np.int32).astype(np.int8)
    up_i8 = np.asarray(up_w, dtype=np.int32).astype(np.int8)
    down_i8 = np.asarray(down_w, dtype=np.int32).astype(np.int8)

    # gate/up interleaved, padded + tiled: [IBP, HK, 128(h), 128|128]
    gp = np.zeros((IP, H), dtype=np.int8)
    gp[:I] = gate_i8
    up_p = np.zeros((IP, H), dtype=np.int8)
    up_p[:I] = up_i8
    g4 = gp.reshape(IBP, 128, HK, 128).transpose(0, 2, 3, 1)
    u4 = up_p.reshape(IBP, 128, HK, 128).transpose(0, 2, 3, 1)
    gu = np.ascontiguousarray(np.concatenate([g4, u4], axis=3))

    dn = np.zeros((IP, H), dtype=np.int8)          # [I_pad, H] = down_w.T
    dn[:I] = down_i8.T

    gb = np.ascontiguousarray(
        np.asarray(gate_b, np.float32).reshape(IB, 128).T)  # [128, IB]
    ub = np.ascontiguousarray(
        np.asarray(up_b, np.float32).reshape(IB, 128).T)
    db = np.ascontiguousarray(
        np.asarray(down_b, np.float32).reshape(HB, 128).T)  # [128, HB]

    hs = np.asarray(hidden_states, dtype=np.int32).astype(np.int8)
    x_parts = []
    for c in range(N_CORES):
        xc = hs[c * TPC: (c + 1) * TPC]                     # [1024, 4096]
        xt = np.ascontiguousarray(xc.T).reshape(HK, 128, TPC)
        xt = np.stack([xt[:, :, ch * TC: (ch + 1) * TC]
                       for ch in range(N_CHUNK)])           # [2, HK, 128, TC]
        x_parts.append(xt)
    x_all = np.ascontiguousarray(np.concatenate(x_parts, axis=0))
    return dict(gu=gu, dn=dn, x8=x_all, gb=gb, ub=ub, db=db)


# --------------------------------------------------------------------------
# Execution via the axon PJRT path, inputs device-resident across iters
# --------------------------------------------------------------------------
def _io_names(nc):
    import jax
    in_names, out_names, out_avals = [], [], []
    for alloc in nc.m.functions[0].allocations:
        if not isinstance(alloc, mybir.MemoryLocationSet):
            continue
        name = alloc.memorylocations[0].name
        if alloc.kind == "ExternalInput":
            in_names.append(name)
        elif alloc.kind == "ExternalOutput":
            out_names.append(name)
            out_avals.append(jax.core.ShapedArray(
                tuple(alloc.tensor_shape), mybir.dt.np(alloc.dtype)))
    return in_names, out_names, out_avals


def _make_call(nc, mesh, n_out_donated):
    """jit(shard_map(bass_exec)) with every tensor sharded on axis 0 and the
    trailing n_out_donated args (output buffers) donated."""
    import jax
    from jax.experimental.shard_map import shard_map
    from jax.sharding import PartitionSpec

    from concourse.bass2jax import _bass_exec_p

    in_names, out_names, out_avals = _io_names(nc)
    all_names = tuple(in_names + out_names)
    n_params = len(in_names)

    def _body(*args):
        outs = _bass_exec_p.bind(
            *args,
            out_avals=tuple(out_avals),
            in_names=all_names,
            out_names=tuple(out_names),
            lowering_input_output_aliases=(),
            sim_require_finite=True,
            sim_require_nnan=True,
            nc=nc,
        )
        return tuple(outs)

    n_args = n_params + len(out_names)
    specs = (PartitionSpec("core"),) * n_args
    donate = tuple(range(n_args - n_out_donated, n_args))
    call = jax.jit(
        shard_map(_body, mesh=mesh, in_specs=specs,
                  out_specs=(PartitionSpec("core"),) * len(out_names),
                  check_rep=False),
        donate_argnums=donate, keep_unused=True)
    return call, in_names, out_names, out_avals


def _get_ntff_hook():
    """NRT profiler hook (dir, device_ids) -> context manager, or raise."""
    from trn_agent_boot.trn_boot import _ntff_profile_via_ctypes

    hook = _ntff_profile_via_ctypes('/opt/axon/libaxon_pjrt.so')
    if hook is None:
        raise RuntimeError("libaxon_pjrt.so lacks NTFF profile symbols")
    return hook


def _gauge_profile(outdir, nc_main):
    import gauge.profiler
    from concourse._compat import FishPath

    return gauge.profiler.Profile(
        profile_path=FishPath(outdir),
        kernel_dev_mode=True,
        profile_on_exit=False,
        bass_kernel=nc_main.m,
        offline_processing=True,
        fname="*_body*",
    )


def _ntff_span_ns(json_path):
    """Instruction-timeline span (max end - min start) from a
    neuron-profile JSON, via streaming parse (fast, no perfetto)."""
    import json as _json

    dec = _json.JSONDecoder()
    tmin, tmax = None, 0
    with open(json_path) as f:
        buf = f.read(2_000_000)
        i = buf.find('"instruction":[')
        if i < 0:
            return 0
        pos = i + len('"instruction":[')
        while True:
            if len(buf) - pos < 200_000:
                more = f.read(4_000_000)
                buf = buf[pos:] + more
                pos = 0
                if not more and len(buf) < 10:
                    break
            while pos < len(buf) and buf[pos] in ', \n\t':
                pos += 1
            if pos >= len(buf) or buf[pos] == ']':
                break
            try:
                obj, end = dec.raw_decode(buf, pos)
            except _json.JSONDecodeError:
                more = f.read(8_000_000)
                if not more:
                    break
                buf = buf[pos:] + more
                pos = 0
                continue
            pos = end
            ts, du = obj.get('timestamp', 0), obj.get('duration', 0)
            if tmin is None or ts < tmin:
                tmin = ts
            if ts + du > tmax:
                tmax = ts + du
    return tmax - (tmin or 0)


def _process_best_ntff(outdirs, nc_main):
    """Among captured NTFF dirs, pick the fastest valid execution by quick
    span extraction, then run the full neuron-profile -> perfetto pipeline
    on it for the official exec_time_ns."""
    import glob

    best_dir, best_span = None, None
    for d in outdirs:
        try:
            if not glob.glob(os.path.join(d, "*_body*.ntff")):
                continue
            prof = _gauge_profile(d, nc_main)
            prof.convert_ntffs_to_json((0,))
            span = _ntff_span_ns(prof.json_path(0).path)
            print(f"[kernel] capture {d}: span {span} ns")
            if span > 1_000_000 and (best_span is None or span < best_span):
                best_dir, best_span = d, span
        except Exception as e:
            print(f"[kernel] capture {d} unusable: {type(e).__name__}: {e}")
    if best_dir is None:
        raise RuntimeError("no valid NTFF capture")

    results = _gauge_profile(best_dir, nc_main).to_perfetto(model_index=(0,))
    if not results or results[0].exec_time_ns is None:
        raise RuntimeError("NTFF processing produced no exec_time_ns")
    print(f"[kernel] perfetto trace: {results[0].trace_path}")
    return int(results[0].exec_time_ns)


def kernel(hidden_states, gate_w, gate_a, gate_b, up_w, up_a, up_b,
           down_w, down_a, down_b):
    global _exec_ns
    import jax
    from jax.sharding import Mesh, NamedSharding, PartitionSpec

    from concourse.bass2jax import install_neuronx_cc_hook
    install_neuronx_cc_hook()

    t0 = time.time()
    host = _prep_inputs(hidden_states, gate_w, gate_b, up_w, up_b,
                        down_w, down_b)
    t_prep = time.time() - t0

    t0 = time.time()
    nc_prep = _build_prep()
    _split_waits(nc_prep)
    nc_main = _build_main(float(np.asarray(gate_a)), float(np.asarray(up_a)),
                          float(np.asarray(down_a)))
    _split_waits(nc_main)
    t_build = time.time() - t0

    devices = jax.devices()[:N_CORES]
    mesh = Mesh(np.asarray(devices), ("core",))
    shard = NamedSharding(mesh, PartitionSpec("core"))

    prep_call, prep_in, prep_out, prep_avals = _make_call(nc_prep, mesh, 3)
    main_call, main_in, main_out, main_avals = _make_call(nc_main, mesh, 1)

    # ---- one-time H2D: int8 weight shards + int8 x + fp32 biases ----
    t0 = time.time()
    host_global = {
        "gus": host["gu"],                     # [IBP, HK, 128, 256] int8
        "dns": host["dn"],                     # [IP, H] int8
        "x8": host["x8"],                      # [16, HK, 128, TC] int8
        "gb": np.tile(host["gb"], (N_CORES, 1)),
        "ub": np.tile(host["ub"], (N_CORES, 1)),
        "db": np.tile(host["db"], (N_CORES, 1)),
    }
    dev = {k: jax.device_put(v, shard) for k, v in host_global.items()}
    jax.block_until_ready(list(dev.values()))
    t_h2d = time.time() - t0

    # device-side zero buffers (no wire traffic)
    def _zeros(aval):
        return jax.jit(
            lambda: jax.numpy.zeros((N_CORES * aval.shape[0],) +
                                    tuple(aval.shape[1:]), aval.dtype),
            out_shardings=shard)()

    # ---- prep NEFF: AllGather + cast (once) ----
    t0 = time.time()
    prep_zeros = [_zeros(a) for a in prep_avals]
    prep_args = [dev[n] for n in prep_in] + prep_zeros
    prep_res = prep_call(*prep_args)
    jax.block_until_ready(prep_res)
    t_prep_neff = time.time() - t0
    prep_map = dict(zip(prep_out, prep_res))

    # ---- main NEFF iterations ----
    main_inputs = {
        "x": prep_map["xbf"], "gu": prep_map["gubf"], "dn": prep_map["dnbf"],
        "gb": dev["gb"], "ub": dev["ub"], "db": dev["db"],
    }
    n_iter = int(os.environ.get("KERNEL_ITERS", "1"))

    t0 = time.time()
    out = main_call(*[main_inputs[n] for n in main_in],
                    _zeros(main_avals[0]))
    jax.block_until_ready(out)
    t_first = time.time() - t0
    print(f"[kernel] host_prep {t_prep:.1f}s  build {t_build:.1f}s  "
          f"H2D {t_h2d:.1f}s  prep_neff(compile+exec) {t_prep_neff:.1f}s  "
          f"main first(compile+exec) {t_first:.1f}s")

    # steady-state timing: chained async dispatches, one block at the end.
    # Executions serialize on device through the donated output buffer, so
    # wall/B is per-execution time with dispatch latency amortized.
    global _exec_wall_ns, _exec_ntff_ns
    if n_iter > 1:
        for _ in range(2):  # warm
            out = main_call(*[main_inputs[n] for n in main_in], out[0])
        jax.block_until_ready(out)
        best = None
        B = int(os.environ.get("KERNEL_BATCH", "128"))
        for _ in range(2):
            t0 = time.time()
            for _ in range(B):
                out = main_call(*[main_inputs[n] for n in main_in], out[0])
            jax.block_until_ready(out)
            avg = (time.time() - t0) / B
            best = avg if best is None else min(best, avg)
        _exec_wall_ns = int(best * 1e9)
        print(f"[kernel] chained-dispatch wall (B={B}): "
              f"{best * 1e3:.3f} ms/iter")

    if os.environ.get("KERNEL_TRACE", "0") == "1":
        try:
            import tempfile
            hook = _get_ntff_hook()
            outdirs = []
            for _ in range(int(os.environ.get("KERNEL_TRACE_N", "2"))):
                outdir = tempfile.mkdtemp(prefix="ntff_")
                with hook(outdir, [0]):
                    out = main_call(*[main_inputs[n] for n in main_in],
                                    out[0])
                    jax.block_until_ready(out)
                outdirs.append(outdir)
            _exec_ntff_ns = _process_best_ntff(outdirs, nc_main)
            print(f"[kernel] NTFF (neuron-profile) HW exec: "
                  f"{_exec_ntff_ns} ns")
        except Exception as e:  # degrade to wall-clock metric
            print(f"[kernel] NTFF capture failed ({type(e).__name__}: {e}); "
                  f"falling back to wall metric")

    _exec_ns = _exec_ntff_ns if _exec_ntff_ns is not None else _exec_wall_ns

    res = np.asarray(out[0]).reshape(N_CORES, H, TPC)
    out_full = np.empty((T, H), dtype=np.float32)
    for c in range(N_CORES):
        out_full[c * TPC: (c + 1) * TPC] = res[c].T
    return out_full
